# revision 26
# baseline (speedup 1.0000x reference)
"""GAT attention kernel for Trainium2 (Bass/Tile), 8-core data parallel.

Per-core math (2 examples each, N=256 items, D=64):
  e   = LayerNorm(emb);  ua = e[0] * e[2:]
  qk  = LeakyReLU(s_q_i + s_k_j + c);  alpha = softmax_j
  attention over value_ij = LN(ua_i * ua_j) collapsed via gram matrices:
    mu = UA@UA^T/D,  E2 = UA^2@UA^2^T/D,  invs = rsqrt(E2 - mu^2)
    att_i = g*((ua_i*St_i - ct_i) * rden_i) + b
  with St = beta~@UA, beta~ = exp(qk)*invs (unnormalized), rden = 1/sum_j exp,
  ct_i = rowsum(ua_i*St_i)/D  (uses mu_ij = ua_i.ua_j/D).
  out = LeakyReLU(concat([e0*e1], att))

Perf structure:
  - beta~ built TRANSPOSED ([j, i] layout) so it feeds S = beta~@UA as lhsT
    directly: no PE transposes of beta, no PSUM->SBUF copies for it.
    qk^T = Prelu(bcast(s_q row) + s_k col bias); softmax denominators via
    ones-vector matmul over partitions; rden transposed back by two tiny
    PE transposes.
  - variance of all 4 (example, block) tiles batched into one [128, 1024]
    tile; inv-sigma = one ACT Abs_reciprocal_sqrt (exactly 2 act-table
    switches per kernel, enforced with explicit deps after the Exps).
  - embedding-LN rsqrt on DVE (quake bit trick + 1 Newton step) to stay in
    the exp act-table set.
  - all matmuls fp32r (PE 2-pass instead of fp32 4-pass).
"""

import numpy as np

import concourse.bass as bass
from concourse import bacc
import concourse.mybir as mybir
import concourse.tile as tile
from concourse import masks
from concourse.bass_utils import run_bass_kernel_spmd
from concourse.tile import add_dep_helper

F32 = mybir.dt.float32
F32R = mybir.dt.float32r
I32 = mybir.dt.int32
ALU = mybir.AluOpType
ACTF = mybir.ActivationFunctionType
AX = mybir.AxisListType

B, NODE, D = 16, 258, 64
N = NODE - 2
N_CORES = 8
B_LOC = B // N_CORES
EPS = 1e-5
SLOPE = 0.01
OUT_ROWS = N + 1
MAGIC = 0x5f375a86


def _rsqrt(nc, pool, x, P, W, pfx):
    """x**-0.5 on DVE: bit trick + 1 Newton iteration. rel err ~1.8e-3."""
    y0 = pool.tile([P, W], F32, tag=pfx + "_y0")
    nc.vector.tensor_scalar(y0.bitcast(I32)[:], x.bitcast(I32)[:], 1, None,
                            op0=ALU.logical_shift_right)
    nc.vector.tensor_scalar(y0.bitcast(I32)[:], y0.bitcast(I32)[:], -1, MAGIC,
                            op0=ALU.mult, op1=ALU.add)
    t = pool.tile([P, W], F32, tag=pfx + "_t")
    nc.vector.tensor_mul(t[:], y0[:], y0[:])
    u = pool.tile([P, W], F32, tag=pfx + "_u")
    nc.vector.scalar_tensor_tensor(u[:], t[:], 0.5, x[:], op0=ALU.mult, op1=ALU.mult)
    v = pool.tile([P, W], F32, tag=pfx + "_v")
    nc.vector.tensor_mul(v[:], u[:], y0[:])
    r = pool.tile([P, W], F32, tag=pfx + "_r")
    nc.vector.scalar_tensor_tensor(r[:], y0[:], 1.5, v[:], op0=ALU.mult, op1=ALU.subtract)
    return r


def _lrelu(nc, out_ap, in_ap):
    nc.vector.scalar_tensor_tensor(out_ap, in_ap, SLOPE, in_ap, op0=ALU.mult, op1=ALU.max)


def build():
    nc = bacc.Bacc()
    emb = nc.dram_tensor("emb", [B_LOC, NODE, D], F32, kind="ExternalInput")
    cstT = nc.dram_tensor("cstT", [D, 2], F32, kind="ExternalInput")   # cols: vq, vk
    cstR = nc.dram_tensor("cstR", [1, 4 * D], F32, kind="ExternalInput")  # [g|b|vi|C0..]
    out = nc.dram_tensor("out", [B_LOC, OUT_ROWS, D], F32, kind="ExternalOutput")

    with tile.TileContext(nc) as tc:
        with (
            tc.tile_pool(name="const", bufs=1) as cpool,
            tc.tile_pool(name="work", bufs=2) as pool,
            tc.tile_pool(name="psmall", bufs=3, space="PSUM") as psmall,
            tc.tile_pool(name="pqk", bufs=1, space="PSUM") as pqk,
            tc.tile_pool(name="pmue2", bufs=2, space="PSUM") as pmue2,
            tc.tile_pool(name="ps", bufs=2, space="PSUM") as ps,
        ):
            # ---- global constants ----
            identF = cpool.tile([128, 128], F32)
            masks.make_identity(nc, identF[:])
            identR = cpool.tile([128, 128], F32R)
            nc.scalar.copy(identR[:], identF[:])
            ones_f = cpool.tile([1, 128], F32)
            nc.vector.memset(ones_f[:], 1.0)
            ones_r = cpool.tile([1, 128], F32R)
            nc.scalar.copy(ones_r[:], ones_f[:])
            ones_cf = cpool.tile([128, 2], F32)
            nc.vector.memset(ones_cf[:], 1.0)
            ones_cr = cpool.tile([128, 2], F32R)
            nc.scalar.copy(ones_cr[:], ones_cf[:])

            # input DMAs first on the sync queue: they gate the pipeline.
            # U rows of both examples land in ONE tile at quadrant-aligned
            # partitions {0,32,64,96} so compute can address each row.
            tU4 = cpool.tile([128, D], F32)
            u4v = tU4[:].rearrange("(a b) d -> a b d", b=32)
            nc.sync.dma_start(u4v[0:2, 0:1, :], emb[0, 0:2, :])
            nc.sync.dma_start(u4v[2:4, 0:1, :], emb[1, 0:2, :])
            # item rows 2..257 as [128, 2, 64], row r = 2p + n
            in_tiles = []
            for e in range(B_LOC):
                tAB = pool.tile([128, 2, D], F32, tag=f"tAB{e}")
                nc.sync.dma_start(tAB[:], emb[e, 2:258, :].rearrange("(p n) d -> p n d", n=2))
                in_tiles.append(tAB)

            cst_sb = cpool.tile([1, 4 * D], F32)
            nc.gpsimd.dma_start(cst_sb[:], cstR[:, :])
            gb_row = cst_sb[:, 0:2 * D]
            vi_row = cst_sb[:, 2 * D:3 * D]
            c0_sb = cst_sb[:, 3 * D:3 * D + 1]
            vqk = cpool.tile([D, 2], F32)
            nc.gpsimd.dma_start(vqk[:], cstT[:, :])
            vqkr = cpool.tile([D, 2], F32R)
            nc.scalar.copy(vqkr[:], vqk[:])
            gb_rowr = cpool.tile([1, 2 * D], F32R)
            nc.scalar.copy(gb_rowr[:], gb_row)

            p_gb = psmall.tile([128, 2 * D], F32, tag="small")
            nc.tensor.matmul(p_gb[:], ones_r[:], gb_rowr[:])
            gb_bc = cpool.tile([128, 2 * D], F32)
            nc.scalar.copy(gb_bc[:], p_gb[:])
            g_bc = gb_bc[:, 0:D]
            b_bc = gb_bc[:, D:2 * D]

            # per-example variance tiles [128, 2N]
            msq_big0 = cpool.tile([128, 2 * N], F32)
            msq_big1 = cpool.tile([128, 2 * N], F32)
            e2s_big0 = cpool.tile([128, 2 * N], F32)
            e2s_big1 = cpool.tile([128, 2 * N], F32)
            msq_bigs = [msq_big0, msq_big1]
            e2s_bigs = [e2s_big0, e2s_big1]
            var_e0 = cpool.tile([128, 2 * N], F32)
            var_e1 = cpool.tile([128, 2 * N], F32)
            rstd_e0 = cpool.tile([128, 2 * N], F32)
            rstd_e1 = cpool.tile([128, 2 * N], F32)
            var_es = [var_e0, var_e1]
            rstd_es = [rstd_e0, rstd_e1]

            st = [dict() for _ in range(B_LOC)]
            exp_insts = []
            rstd_insts = []

            # ---- U rows (both examples): one LN chain on quadrant rows ----
            sum_u = pool.tile([128, 1], F32, tag="sum_u")
            nc.vector.reduce_sum(sum_u[:], tU4[:], axis=AX.X)
            nm_u = pool.tile([128, 1], F32, tag="nm_u")
            nc.vector.tensor_scalar_mul(nm_u[:], sum_u[:], -1.0 / D)
            xcU = pool.tile([128, D], F32, tag="xcU")
            nc.vector.tensor_scalar_add(xcU[:], tU4[:], nm_u[:])
            squ = pool.tile([128, D], F32, tag="squ")
            ss_u = pool.tile([128, 1], F32, tag="ss_u")
            nc.scalar.activation(squ[:], xcU[:], ACTF.Square, accum_out=ss_u[:])
            xvu = pool.tile([128, 1], F32, tag="xvu")
            nc.vector.tensor_scalar(xvu[:], ss_u[:], 1.0 / D, EPS, op0=ALU.mult, op1=ALU.add)
            rstd_u = _rsqrt(nc, pool, xvu, 128, 1, "lnu")
            elnU4 = cpool.tile([128, D], F32R)
            nc.vector.scalar_tensor_tensor(elnU4[:], xcU[:], rstd_u[:],
                                           g_bc, op0=ALU.mult, op1=ALU.mult)
            nc.vector.tensor_add(elnU4[:], elnU4.bitcast(F32)[:], b_bc)

            # ---- A/B LN stats for both examples, batched [128, 4] ----
            sum_b = pool.tile([128, 4], F32, tag="sum_b")
            for e in range(B_LOC):
                nc.vector.reduce_sum(sum_b[:, 2 * e:2 * e + 1], in_tiles[e][:, 0, :], axis=AX.X)
                nc.vector.reduce_sum(sum_b[:, 2 * e + 1:2 * e + 2], in_tiles[e][:, 1, :], axis=AX.X)
            nm_b = pool.tile([128, 4], F32, tag="nm_b")
            nc.vector.tensor_scalar_mul(nm_b[:], sum_b[:], -1.0 / D)
            ss_b = pool.tile([128, 4], F32, tag="ss_b")
            xcs = []
            for e in range(B_LOC):
                for n in range(2):
                    xc = pool.tile([128, D], F32, tag=f"xc{e}{n}")
                    nc.vector.tensor_scalar_add(xc[:], in_tiles[e][:, n, :],
                                                nm_b[:, 2 * e + n:2 * e + n + 1])
                    sqs = pool.tile([128, D], F32, tag=f"sqs{n}")
                    nc.scalar.activation(sqs[:], xc[:], ACTF.Square,
                                         accum_out=ss_b[:, 2 * e + n:2 * e + n + 1])
                    xcs.append(xc)
            xv = pool.tile([128, 4], F32, tag="xv")
            nc.vector.tensor_scalar(xv[:], ss_b[:], 1.0 / D, EPS, op0=ALU.mult, op1=ALU.add)
            rstd_ab = _rsqrt(nc, pool, xv, 128, 4, "lnr")

            # ================= pass A: per-example =========
            for e in range(B_LOC):
                S = st[e]

                iid_row = pool.tile([1, D], F32, tag="iid")
                nc.sync.dma_start(iid_row[:], elnU4.bitcast(F32)[64 * e + 32:64 * e + 33, :])
                S["iid_row"] = iid_row
                S["elnU"] = elnU4

                # uid0 broadcast as early as possible (PE needs operands at
                # the same base partition -> stage e1's uid row at partition 0)
                if e == 0:
                    u0row = elnU4[0:1, :]
                else:
                    u0r = pool.tile([1, D], F32R, tag="u0r")
                    nc.scalar.copy(u0r[:], elnU4.bitcast(F32)[64:65, :])
                    u0row = u0r[:]
                S["u0row"] = u0row
                p_u0 = psmall.tile([128, D], F32, tag="small")
                nc.tensor.matmul(p_u0[:], ones_r[:], u0row)

                ui = pool.tile([1, D], F32, tag="ui")
                nc.vector.tensor_mul(ui[:], u0row.bitcast(F32), iid_row[:])
                uo = pool.tile([1, D], F32, tag="uo")
                _lrelu(nc, uo[:], ui[:])
                (nc.sync if e == 0 else nc.gpsimd).dma_start(out[e, 0:1, :], uo[:])

                G0 = pool.tile([128, D], F32, tag="G0")
                nc.vector.tensor_mul(G0[:], g_bc, p_u0[:])
                B0 = pool.tile([128, D], F32, tag="B0")
                nc.vector.tensor_mul(B0[:], b_bc, p_u0[:])
                ua_both = pool.tile([128, 2, D], F32R, tag="ua_both")
                for n in range(2):
                    uap = pool.tile([128, D], F32, tag=f"uap{n}")
                    nc.vector.scalar_tensor_tensor(uap[:], xcs[2 * e + n][:],
                                                   rstd_ab[:, 2 * e + n:2 * e + n + 1],
                                                   G0[:], op0=ALU.mult, op1=ALU.mult)
                    nc.vector.tensor_add(ua_both[:, n, :], uap[:], B0[:])
                S["ua_both"] = ua_both

                # UA^T and (UA^2)^T
                p_t = psmall.tile([D, N], F32R, tag="small")
                nc.tensor.transpose(p_t[:, 0:128], ua_both[:, 0, :], identR[:])
                nc.tensor.transpose(p_t[:, 128:256], ua_both[:, 1, :], identR[:])
                uat = pool.tile([D, N], F32R, tag="uat")
                nc.scalar.copy(uat[:], p_t.bitcast(F32)[:])
                ua2t = pool.tile([D, N], F32R, tag="ua2t")
                nc.scalar.activation(ua2t[:], uat.bitcast(F32)[:], ACTF.Square)
                S["uat"] = uat
                S["ua2t"] = ua2t

            # ============ pass A2: scores + grams (interleaved engines) ====
            for e in range(B_LOC):
                S = st[e]
                uat = S["uat"]
                ua2t = S["ua2t"]
                iid_row = S["iid_row"]

                # scores: s_k columns per j-block, s_q as a row
                sk_sb = pool.tile([128, 2], F32, tag="sk_sb")
                for J in range(2):
                    cs = slice(J * 128, (J + 1) * 128)
                    p_sqk = psmall.tile([128, 2], F32, tag="small")
                    nc.tensor.matmul(p_sqk[:], uat[:, cs], vqkr[:, 0:2])
                    nc.scalar.copy(sk_sb[:, J:J + 1], p_sqk[:, 1:2])
                S["sk_sb"] = sk_sb

                p_sqrow = psmall.tile([1, N], F32, tag="small")
                nc.tensor.matmul(p_sqrow[:], vqkr[:, 0:1], uat[:])

                si_scr = pool.tile([1, D], F32, tag="si_scr")
                nc.vector.tensor_mul(si_scr[:], iid_row[:], vi_row[:])
                si = pool.tile([1, 1], F32, tag="si")
                nc.vector.reduce_sum(si[:], si_scr[:], axis=AX.X)
                c_all = pool.tile([1, 1], F32, tag="c_all")
                nc.vector.tensor_scalar_add(c_all[:], si[:], c0_sb[:])
                sqc = pool.tile([1, N], F32R, tag="sqc")
                nc.scalar.activation(sqc[:], p_sqrow[:], ACTF.Identity, bias=c_all[:])

                # qk^T = Prelu(bcast(s_q row) + s_k col); exp (unnormalized)
                p_qkT = pqk.tile([128, N], F32, tag="qk")
                nc.tensor.matmul(p_qkT[:], ones_r[:], sqc[:])
                expvTs = []
                for J in range(2):
                    qkT = pool.tile([128, N], F32, tag="qkT")
                    nc.scalar.activation(qkT[:], p_qkT[:], ACTF.Prelu,
                                         bias=sk_sb[:, J:J + 1], alpha=SLOPE)
                    expvT = pool.tile([128, N], F32R, tag=f"expvT{e}{J}")
                    ei = nc.scalar.activation(expvT[:], qkT[:], ACTF.Exp)
                    exp_insts.append(ei)
                    expvTs.append(expvT)
                S["expvTs"] = expvTs

                # softmax denominators, directly as columns:
                # denom[i] = sum_j expT[j, i] via matmul(lhsT=expT[:, iblk], rhs=ones)
                rden_cols = pool.tile([128, 2], F32, tag=f"rdenc{e}")
                for blk in range(2):
                    cs = slice(blk * 128, (blk + 1) * 128)
                    p_denc = psmall.tile([128, 2], F32, tag="small")
                    nc.tensor.matmul(p_denc[:], expvTs[0][:, cs], ones_cr[:],
                                     start=True, stop=False)
                    nc.tensor.matmul(p_denc[:], expvTs[1][:, cs], ones_cr[:],
                                     start=False, stop=True)
                    nc.vector.reciprocal(rden_cols[:, blk:blk + 1], p_denc[:, 0:1])
                rg0 = pool.tile([128, D], F32, tag="rg0")
                nc.vector.tensor_scalar_mul(rg0[:], g_bc, rden_cols[:, 0:1])
                rg1 = pool.tile([128, D], F32, tag="rg1")
                nc.vector.tensor_scalar_mul(rg1[:], g_bc, rden_cols[:, 1:2])
                S["rgs"] = [rg0, rg1]

                # gram matrices -> scaled mu^2 and E2/D slices of the big tiles
                for blk in range(2):
                    cs = slice(blk * 128, (blk + 1) * 128)
                    ns = slice(blk * N, (blk + 1) * N)
                    p_mu = pmue2.tile([128, N], F32, tag="mue2")
                    nc.tensor.matmul(p_mu[:], uat[:, cs], uat[:])
                    p_e2 = pmue2.tile([128, N], F32, tag="mue2")
                    nc.tensor.matmul(p_e2[:], ua2t[:, cs], ua2t[:])
                    nc.scalar.activation(msq_bigs[e][:, ns], p_mu[:], ACTF.Square, scale=1.0 / D)
                    nc.scalar.mul(e2s_bigs[e][:, ns], p_e2[:], 1.0 / D)

                # per-example inv-sigma: unblocks this example's pass B without
                # waiting for the other example's gram matmuls
                var_e = var_es[e]
                nc.vector.tensor_sub(var_e[:], e2s_bigs[e][:], msq_bigs[e][:])
                rstd_e = rstd_es[e]
                ri = nc.scalar.activation(rstd_e[:], var_e[:], ACTF.Abs_reciprocal_sqrt)
                rstd_insts.append(ri)
                st[e]["rstd"] = rstd_e

            for ri in rstd_insts:
                for ei in exp_insts:
                    add_dep_helper(ri.ins, ei.ins, sync=False,
                                   reason="abs-rsqrt after all exp-set ACT ops")

            # ================= pass B: attention + output =================
            for e in range(B_LOC):
                S = st[e]
                ua_both = S["ua_both"]

                btTs = []
                for J in range(2):
                    ns = slice(J * N, (J + 1) * N)
                    btT = pool.tile([128, N], F32R, tag=f"btT{J}")
                    nc.vector.tensor_mul(btT[:], S["expvTs"][J].bitcast(F32)[:],
                                         S["rstd"][:, ns])
                    btTs.append(btT)

                p_S2 = ps.tile([128, 2, D], F32, tag="S")
                for blk in range(2):
                    cs = slice(blk * 128, (blk + 1) * 128)
                    nc.tensor.matmul(p_S2[:, blk, :], btTs[0][:, cs], ua_both[:, 0, :],
                                     start=True, stop=False)
                    nc.tensor.matmul(p_S2[:, blk, :], btTs[1][:, cs], ua_both[:, 1, :],
                                     start=False, stop=True)

                t1b = pool.tile([128, 2, D], F32, tag="t1b")
                nc.vector.tensor_mul(t1b[:], ua_both.bitcast(F32)[:], p_S2[:])
                c_raw = pool.tile([128, 2], F32, tag="c_raw")
                nc.vector.reduce_sum(c_raw[:], t1b[:], axis=AX.X)
                c_col = pool.tile([128, 2], F32, tag="c_col")
                nc.vector.tensor_scalar_mul(c_col[:], c_raw[:], 1.0 / D)

                o_big = pool.tile([128, 2, D], F32, tag="o_big")
                out_rows = out[e, 1:257, :].rearrange("(p n) d -> p n d", n=2)
                for blk in range(2):
                    t2 = pool.tile([128, D], F32, tag="t2")
                    nc.vector.scalar_tensor_tensor(t2[:], t1b[:, blk, :], c_col[:, blk:blk + 1],
                                                   S["rgs"][blk][:], op0=ALU.subtract, op1=ALU.mult)
                    t3 = pool.tile([128, D], F32, tag="t3")
                    nc.vector.tensor_add(t3[:], t2[:], b_bc)
                    _lrelu(nc, o_big[:, blk, :], t3[:])
                    (nc.sync if e == 0 else nc.gpsimd).dma_start(
                        out_rows[:, blk:blk + 1, :], o_big[:, blk:blk + 1, :])

    nc.compile()
    return nc


def _host_consts(Wa, ba, a_w, a_b):
    aq, ak, ai = a_w[:D], a_w[D:2 * D], a_w[2 * D:]
    vq = aq @ Wa
    vk = ak @ Wa
    vi = ai @ Wa
    c0 = float(ba @ aq + ba @ ak + ba @ ai + a_b[0])
    cstT = np.stack([vq, vk], axis=1).astype(np.float32)
    cstR = np.zeros((1, 4 * D), np.float32)
    cstR[0, 2 * D:3 * D] = vi
    cstR[0, 3 * D] = c0
    return cstT, cstR


_NC_CACHE = {}


def _get_nc():
    if "nc" not in _NC_CACHE:
        _NC_CACHE["nc"] = build()
    return _NC_CACHE["nc"]


def run(embeddings, Wa, ba, a_w, a_b, ln_g, ln_b, **spmd_kwargs):
    embeddings = np.ascontiguousarray(embeddings, dtype=np.float32)
    cstT, cstR = _host_consts(np.asarray(Wa, np.float32), np.asarray(ba, np.float32),
                              np.asarray(a_w, np.float32), np.asarray(a_b, np.float32))
    cstR[0, 0:D] = np.asarray(ln_g, np.float32)
    cstR[0, D:2 * D] = np.asarray(ln_b, np.float32)

    nc = _get_nc()
    in_maps = [
        {"emb": embeddings[c * B_LOC:(c + 1) * B_LOC], "cstT": cstT, "cstR": cstR}
        for c in range(N_CORES)
    ]
    res = run_bass_kernel_spmd(nc, in_maps, core_ids=list(range(N_CORES)), **spmd_kwargs)
    outp = np.concatenate([res.results[c]["out"] for c in range(N_CORES)], axis=0)
    return outp, res


def kernel(embeddings, Wa, ba, a_w, a_b, ln_g, ln_b):
    outp, _ = run(embeddings, Wa, ba, a_w, a_b, ln_g, ln_b)
    return outp


# revision 27
# speedup vs baseline: 1.0309x; 1.0309x over previous
"""GAT attention kernel for Trainium2 (Bass/Tile), 8-core data parallel.

Per-core math (2 examples each, N=256 items, D=64):
  e   = LayerNorm(emb);  ua = e[0] * e[2:]
  qk  = LeakyReLU(s_q_i + s_k_j + c);  alpha = softmax_j
  attention over value_ij = LN(ua_i * ua_j) collapsed via gram matrices:
    mu = UA@UA^T/D,  E2 = UA^2@UA^2^T/D,  invs = rsqrt(E2 - mu^2)
    att_i = g*((ua_i*St_i - ct_i) * rden_i) + b
  with St = beta~@UA, beta~ = exp(qk)*invs (unnormalized), rden = 1/sum_j exp,
  ct_i = rowsum(ua_i*St_i)/D  (uses mu_ij = ua_i.ua_j/D).
  out = LeakyReLU(concat([e0*e1], att))

Perf structure:
  - beta~ built TRANSPOSED ([j, i] layout) so it feeds S = beta~@UA as lhsT
    directly: no PE transposes of beta, no PSUM->SBUF copies for it.
    qk^T = Prelu(bcast(s_q row) + s_k col bias); softmax denominators via
    ones-vector matmul over partitions; rden transposed back by two tiny
    PE transposes.
  - variance of all 4 (example, block) tiles batched into one [128, 1024]
    tile; inv-sigma = one ACT Abs_reciprocal_sqrt (exactly 2 act-table
    switches per kernel, enforced with explicit deps after the Exps).
  - embedding-LN rsqrt on DVE (quake bit trick + 1 Newton step) to stay in
    the exp act-table set.
  - all matmuls fp32r (PE 2-pass instead of fp32 4-pass).
"""

import numpy as np

import concourse.bass as bass
from concourse import bacc
import concourse.mybir as mybir
import concourse.tile as tile
from concourse import masks
from concourse.bass_utils import run_bass_kernel_spmd
from concourse.tile import add_dep_helper

F32 = mybir.dt.float32
F32R = mybir.dt.float32r
I32 = mybir.dt.int32
ALU = mybir.AluOpType
ACTF = mybir.ActivationFunctionType
AX = mybir.AxisListType

B, NODE, D = 16, 258, 64
N = NODE - 2
N_CORES = 8
B_LOC = B // N_CORES
EPS = 1e-5
SLOPE = 0.01
OUT_ROWS = N + 1
MAGIC = 0x5f375a86


def _rsqrt(nc, pool, x, P, W, pfx):
    """x**-0.5 on DVE: bit trick + 1 Newton iteration. rel err ~1.8e-3."""
    y0 = pool.tile([P, W], F32, tag=pfx + "_y0")
    nc.vector.tensor_scalar(y0.bitcast(I32)[:], x.bitcast(I32)[:], 1, None,
                            op0=ALU.logical_shift_right)
    nc.vector.tensor_scalar(y0.bitcast(I32)[:], y0.bitcast(I32)[:], -1, MAGIC,
                            op0=ALU.mult, op1=ALU.add)
    t = pool.tile([P, W], F32, tag=pfx + "_t")
    nc.vector.tensor_mul(t[:], y0[:], y0[:])
    u = pool.tile([P, W], F32, tag=pfx + "_u")
    nc.vector.scalar_tensor_tensor(u[:], t[:], 0.5, x[:], op0=ALU.mult, op1=ALU.mult)
    v = pool.tile([P, W], F32, tag=pfx + "_v")
    nc.vector.tensor_mul(v[:], u[:], y0[:])
    r = pool.tile([P, W], F32, tag=pfx + "_r")
    nc.vector.scalar_tensor_tensor(r[:], y0[:], 1.5, v[:], op0=ALU.mult, op1=ALU.subtract)
    return r


def _lrelu(nc, out_ap, in_ap):
    nc.vector.scalar_tensor_tensor(out_ap, in_ap, SLOPE, in_ap, op0=ALU.mult, op1=ALU.max)


def build():
    nc = bacc.Bacc()
    emb = nc.dram_tensor("emb", [B_LOC, NODE, D], F32, kind="ExternalInput")
    cstT = nc.dram_tensor("cstT", [D, 2], F32, kind="ExternalInput")   # cols: vq, vk
    cstR = nc.dram_tensor("cstR", [1, 4 * D], F32, kind="ExternalInput")  # [g|b|vi|C0..]
    out = nc.dram_tensor("out", [B_LOC, OUT_ROWS, D], F32, kind="ExternalOutput")

    with tile.TileContext(nc) as tc:
        with (
            tc.tile_pool(name="const", bufs=1) as cpool,
            tc.tile_pool(name="work", bufs=2) as pool,
            tc.tile_pool(name="psmall", bufs=3, space="PSUM") as psmall,
            tc.tile_pool(name="pqk", bufs=1, space="PSUM") as pqk,
            tc.tile_pool(name="pmue2", bufs=2, space="PSUM") as pmue2,
            tc.tile_pool(name="ps", bufs=2, space="PSUM") as ps,
        ):
            # ---- global constants ----
            identF = cpool.tile([128, 128], F32)
            masks.make_identity(nc, identF[:])
            identR = cpool.tile([128, 128], F32R)
            nc.scalar.copy(identR[:], identF[:])
            ones_f = cpool.tile([1, 128], F32)
            nc.vector.memset(ones_f[:], 1.0)
            ones_r = cpool.tile([1, 128], F32R)
            nc.scalar.copy(ones_r[:], ones_f[:])
            ones_cf = cpool.tile([128, 2], F32)
            nc.vector.memset(ones_cf[:], 1.0)
            ones_cr = cpool.tile([128, 2], F32R)
            nc.scalar.copy(ones_cr[:], ones_cf[:])

            # input DMAs first on the sync queue: they gate the pipeline.
            # U rows of both examples land in ONE tile at quadrant-aligned
            # partitions {0,32,64,96} so compute can address each row.
            tU4 = cpool.tile([128, D], F32)
            u4v = tU4[:].rearrange("(a b) d -> a b d", b=32)
            nc.sync.dma_start(u4v[0:2, 0:1, :], emb[0, 0:2, :])
            nc.sync.dma_start(u4v[2:4, 0:1, :], emb[1, 0:2, :])
            # item rows 2..257 as [128, 2, 64], row r = 2p + n
            in_tiles = []
            for e in range(B_LOC):
                tAB = pool.tile([128, 2, D], F32, tag=f"tAB{e}")
                nc.sync.dma_start(tAB[:], emb[e, 2:258, :].rearrange("(p n) d -> p n d", n=2))
                in_tiles.append(tAB)

            cst_sb = cpool.tile([1, 4 * D], F32)
            nc.gpsimd.dma_start(cst_sb[:], cstR[:, :])
            gb_row = cst_sb[:, 0:2 * D]
            vi_row = cst_sb[:, 2 * D:3 * D]
            c0_sb = cst_sb[:, 3 * D:3 * D + 1]
            vqk = cpool.tile([D, 2], F32)
            nc.gpsimd.dma_start(vqk[:], cstT[:, :])
            vqkr = cpool.tile([D, 2], F32R)
            nc.scalar.copy(vqkr[:], vqk[:])
            gb_rowr = cpool.tile([1, 2 * D], F32R)
            nc.scalar.copy(gb_rowr[:], gb_row)

            p_gb = psmall.tile([128, 2 * D], F32, tag="small")
            nc.tensor.matmul(p_gb[:], ones_r[:], gb_rowr[:])
            gb_bc = cpool.tile([128, 2 * D], F32)
            nc.scalar.copy(gb_bc[:], p_gb[:])
            g_bc = gb_bc[:, 0:D]
            b_bc = gb_bc[:, D:2 * D]

            # per-example variance tiles [128, 2N]
            msq_big0 = cpool.tile([128, 2 * N], F32)
            msq_big1 = cpool.tile([128, 2 * N], F32)
            e2s_big0 = cpool.tile([128, 2 * N], F32)
            e2s_big1 = cpool.tile([128, 2 * N], F32)
            msq_bigs = [msq_big0, msq_big1]
            e2s_bigs = [e2s_big0, e2s_big1]
            var_e0 = cpool.tile([128, 2 * N], F32)
            var_e1 = cpool.tile([128, 2 * N], F32)
            rstd_e0 = cpool.tile([128, 2 * N], F32)
            rstd_e1 = cpool.tile([128, 2 * N], F32)
            var_es = [var_e0, var_e1]
            rstd_es = [rstd_e0, rstd_e1]

            st = [dict() for _ in range(B_LOC)]
            exp_insts = []
            rstd_insts = []

            # ---- U rows (both examples): one LN chain on quadrant rows ----
            sum_u = pool.tile([128, 1], F32, tag="sum_u")
            nc.vector.reduce_sum(sum_u[:], tU4[:], axis=AX.X)
            nm_u = pool.tile([128, 1], F32, tag="nm_u")
            nc.vector.tensor_scalar_mul(nm_u[:], sum_u[:], -1.0 / D)
            xcU = pool.tile([128, D], F32, tag="xcU")
            nc.vector.tensor_scalar_add(xcU[:], tU4[:], nm_u[:])
            squ = pool.tile([128, D], F32, tag="squ")
            ss_u = pool.tile([128, 1], F32, tag="ss_u")
            nc.scalar.activation(squ[:], xcU[:], ACTF.Square, accum_out=ss_u[:])
            xvu = pool.tile([128, 1], F32, tag="xvu")
            nc.vector.tensor_scalar(xvu[:], ss_u[:], 1.0 / D, EPS, op0=ALU.mult, op1=ALU.add)
            rstd_u = _rsqrt(nc, pool, xvu, 128, 1, "lnu")
            elnU4 = cpool.tile([128, D], F32R)
            nc.vector.scalar_tensor_tensor(elnU4[:], xcU[:], rstd_u[:],
                                           g_bc, op0=ALU.mult, op1=ALU.mult)
            nc.vector.tensor_add(elnU4[:], elnU4.bitcast(F32)[:], b_bc)

            # ---- A/B LN stats for both examples, batched [128, 4] ----
            sum_b = pool.tile([128, 4], F32, tag="sum_b")
            for e in range(B_LOC):
                nc.vector.reduce_sum(sum_b[:, 2 * e:2 * e + 1], in_tiles[e][:, 0, :], axis=AX.X)
                nc.vector.reduce_sum(sum_b[:, 2 * e + 1:2 * e + 2], in_tiles[e][:, 1, :], axis=AX.X)
            nm_b = pool.tile([128, 4], F32, tag="nm_b")
            nc.vector.tensor_scalar_mul(nm_b[:], sum_b[:], -1.0 / D)
            ss_b = pool.tile([128, 4], F32, tag="ss_b")
            xcs = []
            for e in range(B_LOC):
                for n in range(2):
                    xc = pool.tile([128, D], F32, tag=f"xc{e}{n}")
                    nc.vector.tensor_scalar_add(xc[:], in_tiles[e][:, n, :],
                                                nm_b[:, 2 * e + n:2 * e + n + 1])
                    sqs = pool.tile([128, D], F32, tag=f"sqs{n}")
                    nc.scalar.activation(sqs[:], xc[:], ACTF.Square,
                                         accum_out=ss_b[:, 2 * e + n:2 * e + n + 1])
                    xcs.append(xc)
            xv = pool.tile([128, 4], F32, tag="xv")
            nc.vector.tensor_scalar(xv[:], ss_b[:], 1.0 / D, EPS, op0=ALU.mult, op1=ALU.add)
            rstd_ab = _rsqrt(nc, pool, xv, 128, 4, "lnr")

            # ================= pass A: per-example =========
            for e in range(B_LOC):
                S = st[e]

                iid_row = pool.tile([1, D], F32, tag="iid")
                nc.sync.dma_start(iid_row[:], elnU4.bitcast(F32)[64 * e + 32:64 * e + 33, :])
                S["iid_row"] = iid_row
                S["elnU"] = elnU4

                # uid0 broadcast as early as possible (PE needs operands at
                # the same base partition -> stage e1's uid row at partition 0)
                if e == 0:
                    u0row = elnU4[0:1, :]
                else:
                    u0r = pool.tile([1, D], F32R, tag="u0r")
                    nc.scalar.copy(u0r[:], elnU4.bitcast(F32)[64:65, :])
                    u0row = u0r[:]
                S["u0row"] = u0row
                p_u0 = psmall.tile([128, D], F32, tag="small")
                nc.tensor.matmul(p_u0[:], ones_r[:], u0row)

                elnA = pool.tile([128, D], F32, tag="elnA")
                nc.vector.scalar_tensor_tensor(elnA[:], xcs[2 * e][:], rstd_ab[:, 2 * e:2 * e + 1],
                                               g_bc, op0=ALU.mult, op1=ALU.mult)
                nc.vector.tensor_add(elnA[:], elnA[:], b_bc)
                elnB = pool.tile([128, D], F32, tag="elnB")
                nc.vector.scalar_tensor_tensor(elnB[:], xcs[2 * e + 1][:], rstd_ab[:, 2 * e + 1:2 * e + 2],
                                               g_bc, op0=ALU.mult, op1=ALU.mult)
                nc.vector.tensor_add(elnB[:], elnB[:], b_bc)
                ua_both = pool.tile([128, 2, D], F32R, tag="ua_both")
                nc.vector.tensor_mul(ua_both[:, 0, :], elnA[:], p_u0[:])
                nc.vector.tensor_mul(ua_both[:, 1, :], elnB[:], p_u0[:])
                S["ua_both"] = ua_both

                # UA^T and (UA^2)^T
                p_t = psmall.tile([D, N], F32R, tag="small")
                nc.tensor.transpose(p_t[:, 0:128], ua_both[:, 0, :], identR[:])
                nc.tensor.transpose(p_t[:, 128:256], ua_both[:, 1, :], identR[:])
                uat = pool.tile([D, N], F32R, tag="uat")
                nc.scalar.copy(uat[:], p_t.bitcast(F32)[:])
                ua2t = pool.tile([D, N], F32R, tag="ua2t")
                nc.scalar.activation(ua2t[:], uat.bitcast(F32)[:], ACTF.Square)
                S["uat"] = uat
                S["ua2t"] = ua2t

            # ============ pass A2: scores + grams (interleaved engines) ====
            for e in range(B_LOC):
                S = st[e]
                uat = S["uat"]
                ua2t = S["ua2t"]
                iid_row = S["iid_row"]

                # scores: s_k columns per j-block, s_q as a row
                sk_sb = pool.tile([128, 2], F32, tag="sk_sb")
                for J in range(2):
                    cs = slice(J * 128, (J + 1) * 128)
                    p_sqk = psmall.tile([128, 2], F32, tag="small")
                    nc.tensor.matmul(p_sqk[:], uat[:, cs], vqkr[:, 0:2])
                    nc.scalar.copy(sk_sb[:, J:J + 1], p_sqk[:, 1:2])
                S["sk_sb"] = sk_sb

                p_sqrow = psmall.tile([1, N], F32, tag="small")
                nc.tensor.matmul(p_sqrow[:], vqkr[:, 0:1], uat[:])

                si_scr = pool.tile([1, D], F32, tag="si_scr")
                nc.vector.tensor_mul(si_scr[:], iid_row[:], vi_row[:])
                si = pool.tile([1, 1], F32, tag="si")
                nc.vector.reduce_sum(si[:], si_scr[:], axis=AX.X)
                c_all = pool.tile([1, 1], F32, tag="c_all")
                nc.vector.tensor_scalar_add(c_all[:], si[:], c0_sb[:])
                sqc = pool.tile([1, N], F32R, tag="sqc")
                nc.scalar.activation(sqc[:], p_sqrow[:], ACTF.Identity, bias=c_all[:])

                # qk^T = Prelu(bcast(s_q row) + s_k col); exp (unnormalized)
                p_qkT = pqk.tile([128, N], F32, tag="qk")
                nc.tensor.matmul(p_qkT[:], ones_r[:], sqc[:])
                expvTs = []
                for J in range(2):
                    qkT = pool.tile([128, N], F32, tag="qkT")
                    nc.scalar.activation(qkT[:], p_qkT[:], ACTF.Prelu,
                                         bias=sk_sb[:, J:J + 1], alpha=SLOPE)
                    expvT = pool.tile([128, N], F32R, tag=f"expvT{e}{J}")
                    ei = nc.scalar.activation(expvT[:], qkT[:], ACTF.Exp)
                    exp_insts.append(ei)
                    expvTs.append(expvT)
                S["expvTs"] = expvTs

                # softmax denominators, directly as columns:
                # denom[i] = sum_j expT[j, i] via matmul(lhsT=expT[:, iblk], rhs=ones)
                rden_cols = pool.tile([128, 2], F32, tag=f"rdenc{e}")
                for blk in range(2):
                    cs = slice(blk * 128, (blk + 1) * 128)
                    p_denc = psmall.tile([128, 2], F32, tag="small")
                    nc.tensor.matmul(p_denc[:], expvTs[0][:, cs], ones_cr[:],
                                     start=True, stop=False)
                    nc.tensor.matmul(p_denc[:], expvTs[1][:, cs], ones_cr[:],
                                     start=False, stop=True)
                    nc.vector.reciprocal(rden_cols[:, blk:blk + 1], p_denc[:, 0:1])
                S["rden_cols"] = rden_cols

                # gram matrices -> scaled mu^2 and E2/D slices of the big tiles
                for blk in range(2):
                    cs = slice(blk * 128, (blk + 1) * 128)
                    ns = slice(blk * N, (blk + 1) * N)
                    p_mu = pmue2.tile([128, N], F32, tag="mue2")
                    nc.tensor.matmul(p_mu[:], uat[:, cs], uat[:])
                    p_e2 = pmue2.tile([128, N], F32, tag="mue2")
                    nc.tensor.matmul(p_e2[:], ua2t[:, cs], ua2t[:])
                    nc.scalar.activation(msq_bigs[e][:, ns], p_mu[:], ACTF.Square, scale=1.0 / D)
                    nc.scalar.mul(e2s_bigs[e][:, ns], p_e2[:], 1.0 / D)

                # per-example inv-sigma: unblocks this example's pass B without
                # waiting for the other example's gram matmuls
                var_e = var_es[e]
                nc.vector.tensor_sub(var_e[:], e2s_bigs[e][:], msq_bigs[e][:])
                rstd_e = rstd_es[e]
                ri = nc.scalar.activation(rstd_e[:], var_e[:], ACTF.Abs_reciprocal_sqrt)
                rstd_insts.append(ri)
                st[e]["rstd"] = rstd_e

            for ri in rstd_insts:
                for ei in exp_insts:
                    add_dep_helper(ri.ins, ei.ins, sync=False,
                                   reason="abs-rsqrt after all exp-set ACT ops")

            # ================= pass B: attention + output =================
            for e in range(B_LOC):
                S = st[e]
                ua_both = S["ua_both"]

                btTs = []
                for J in range(2):
                    ns = slice(J * N, (J + 1) * N)
                    btT = pool.tile([128, N], F32R, tag=f"btT{J}")
                    nc.vector.tensor_mul(btT[:], S["expvTs"][J].bitcast(F32)[:],
                                         S["rstd"][:, ns])
                    btTs.append(btT)

                p_S2 = ps.tile([128, 2, D], F32, tag="S")
                for blk in range(2):
                    cs = slice(blk * 128, (blk + 1) * 128)
                    nc.tensor.matmul(p_S2[:, blk, :], btTs[0][:, cs], ua_both[:, 0, :],
                                     start=True, stop=False)
                    nc.tensor.matmul(p_S2[:, blk, :], btTs[1][:, cs], ua_both[:, 1, :],
                                     start=False, stop=True)

                t1b = pool.tile([128, 2, D], F32, tag="t1b")
                nc.vector.tensor_mul(t1b[:], ua_both.bitcast(F32)[:], p_S2[:])
                c_raw = pool.tile([128, 2], F32, tag="c_raw")
                nc.vector.reduce_sum(c_raw[:], t1b[:], axis=AX.X)
                c_col = pool.tile([128, 2], F32, tag="c_col")
                nc.scalar.mul(c_col[:], c_raw[:], 1.0 / D)

                o_big = pool.tile([128, 2, D], F32, tag="o_big")
                for blk in range(2):
                    rg = pool.tile([128, D], F32, tag="rg")
                    nc.vector.tensor_scalar_mul(rg[:], g_bc, S["rden_cols"][:, blk:blk + 1])
                    t2 = pool.tile([128, D], F32, tag="t2")
                    nc.vector.scalar_tensor_tensor(t2[:], t1b[:, blk, :], c_col[:, blk:blk + 1],
                                                   rg[:], op0=ALU.subtract, op1=ALU.mult)
                    t3 = pool.tile([128, D], F32, tag="t3")
                    nc.vector.tensor_add(t3[:], t2[:], b_bc)
                    _lrelu(nc, o_big[:, blk, :], t3[:])
                out_rows = out[e, 1:257, :].rearrange("(p n) d -> p n d", n=2)
                (nc.sync if e == 0 else nc.gpsimd).dma_start(out_rows, o_big[:])

                ui = pool.tile([1, D], F32, tag="ui")
                nc.vector.tensor_mul(ui[:], S["u0row"].bitcast(F32), S["iid_row"][:])
                uo = pool.tile([1, D], F32, tag="uo")
                _lrelu(nc, uo[:], ui[:])
                (nc.sync if e == 0 else nc.gpsimd).dma_start(out[e, 0:1, :], uo[:])

    nc.compile()
    return nc


def _host_consts(Wa, ba, a_w, a_b):
    aq, ak, ai = a_w[:D], a_w[D:2 * D], a_w[2 * D:]
    vq = aq @ Wa
    vk = ak @ Wa
    vi = ai @ Wa
    c0 = float(ba @ aq + ba @ ak + ba @ ai + a_b[0])
    cstT = np.stack([vq, vk], axis=1).astype(np.float32)
    cstR = np.zeros((1, 4 * D), np.float32)
    cstR[0, 2 * D:3 * D] = vi
    cstR[0, 3 * D] = c0
    return cstT, cstR


_NC_CACHE = {}


def _get_nc():
    if "nc" not in _NC_CACHE:
        _NC_CACHE["nc"] = build()
    return _NC_CACHE["nc"]


def run(embeddings, Wa, ba, a_w, a_b, ln_g, ln_b, **spmd_kwargs):
    embeddings = np.ascontiguousarray(embeddings, dtype=np.float32)
    cstT, cstR = _host_consts(np.asarray(Wa, np.float32), np.asarray(ba, np.float32),
                              np.asarray(a_w, np.float32), np.asarray(a_b, np.float32))
    cstR[0, 0:D] = np.asarray(ln_g, np.float32)
    cstR[0, D:2 * D] = np.asarray(ln_b, np.float32)

    nc = _get_nc()
    in_maps = [
        {"emb": embeddings[c * B_LOC:(c + 1) * B_LOC], "cstT": cstT, "cstR": cstR}
        for c in range(N_CORES)
    ]
    res = run_bass_kernel_spmd(nc, in_maps, core_ids=list(range(N_CORES)), **spmd_kwargs)
    outp = np.concatenate([res.results[c]["out"] for c in range(N_CORES)], axis=0)
    return outp, res


def kernel(embeddings, Wa, ba, a_w, a_b, ln_g, ln_b):
    outp, _ = run(embeddings, Wa, ba, a_w, a_b, ln_g, ln_b)
    return outp
